# revision 1
# baseline (speedup 1.0000x reference)
"""Trainium2 Bass kernel v2 for dilated local attention (nn_DilateAttention).

Problem: x [8, 64, 64, 256] f32, W_qkv [768, 256] f32.
  qkv = x @ W_qkv.T; per pixel, per head (8 heads x 32 dim): attention over
  the 9 dilated (3x3, dilation 3) spatial neighbors with zero padding.

Strategy (data-parallel over batch, 1 image per core), [c, m] on-chip layout:
  - PE: f32r transposes of x/W, f32r qkv projection, per-head score
    reduction with product-as-stationary matmuls, and AV accumulation via
    transpose-accumulate matmuls producing rows-layout output directly.
  - DVE/Pool: the q*k / attn*v elementwise products (bf16, SBUF-only so
    Pool is legal); DVE also runs the softmax chain batched 4 m-subs at a
    time on a single-bank [128, 4, 72] PSUM score tile.
  - Attention normalization (1/den) is applied in [m, 72] layout before
    transposing (free-broadcast scalar_tensor_tensor), then the normalized
    attention is transposed to [72, m] and round-tripped through a DRAM
    scratch so the per-channel broadcast becomes a partition-replicating
    (stride-0) DMA read - no PE/PSUM broadcast or evacuation needed.
  - Emission is software-pipelined: chunk ch+1's products/scores/softmax
    are emitted before chunk ch's AV phase so the in-order engine queues
    interleave the two chunks.
"""

import sys

sys.path.insert(0, "/opt/trn_rl_repo")

import numpy as np
import ml_dtypes
from contextlib import ExitStack

import concourse.bass as bass
import concourse.bacc as bacc
import concourse.tile as tile
from concourse import mybir
from concourse.bass_utils import run_bass_kernel_spmd

B, H, W, C = 8, 64, 64, 256
NH, DPH, K2 = 8, 32, 9
N = H * W          # 4096 pixels
PAD = 256          # zero border on each side of k/v (covers |delta| <= 195)
MCH = 1024         # pixels per m-chunk
NCH = N // MCH     # 4 chunks
SUBS = MCH // 128  # 8 m-subs per chunk
NSUB = N // 128    # 32 m-subs
SCALE = DPH ** -0.5
F32 = mybir.dt.float32
F32R = mybir.dt.float32r
BF16 = mybir.dt.bfloat16
NPBF16 = ml_dtypes.bfloat16

DELTAS = [64 * (3 * i - 3) + (3 * j - 3) for i in range(3) for j in range(3)]


def _prod_engine(phase, ch, j, kk):
    # Pool products are SBUF-only (HW-legal) and cheap in-model.
    if phase == 'qk':
        if kk in (1, 4, 7) or (kk in (0, 3) and j == 1):
            return 'pool'
        return 'dve'
    if kk in (1, 4, 7):
        return 'pool'
    return 'dve'


def _host_consts():
    ident = np.eye(128, dtype=np.float32)
    identb = np.eye(128, dtype=NPBF16)
    # score reduce (moving operand): ones_s[p, hh] = 1 iff p//32 == hh
    ones_s = np.zeros((128, 4), NPBF16)
    for p in range(128):
        ones_s[p, p // 32] = 1.0
    # kept for interface compat (unused when broadcast is DMA-only)
    bkk = np.zeros((72, 2, 9, 128), NPBF16)
    for jj in range(2):
        for kk in range(9):
            for q in range(128):
                bkk[(4 * jj + q // 32) * 9 + kk, jj, kk, q] = 1.0
    bkk = bkk.reshape(72, 2 * 9 * 128)
    # column-validity 0/1 mask in [m-sub, f=h*9+kk] layout, plus invalid
    # counts for the denominator (reference zero-pads keys: invalid slots
    # contribute exp(0)=1 to the denominator and 0 to the numerator).
    maskA = np.zeros((128, NSUB, 72), NPBF16)
    cntT = np.zeros((128, NSUB), np.float32)
    for ms in range(NSUB):
        m = ms * 128 + np.arange(128)
        jm = m % 64
        for kk in range(9):
            dc = 3 * (kk % 3) - 3
            valid = (((jm + dc) >= 0) & ((jm + dc) < 64)).astype(np.float32)
            for h in range(8):
                maskA[:, ms, h * 9 + kk] = valid
            cntT[:, ms] += 1.0 - valid
    maskA = maskA.reshape(128, NSUB * 72)
    return ident, identb, ones_s, bkk, maskA, cntT


def build_nc() -> bass.Bass:
    nc = bacc.Bacc()
    x_d = nc.declare_dram_parameter("x", [N, C], F32, isOutput=False)
    w_d = nc.declare_dram_parameter("w", [3 * C, C], F32, isOutput=False)
    ident_d = nc.declare_dram_parameter("ident", [128, 128], F32, isOutput=False)
    identb_d = nc.declare_dram_parameter("identb", [128, 128], BF16, isOutput=False)
    ones_s_d = nc.declare_dram_parameter("ones_s", [128, 4], BF16, isOutput=False)
    bkk_d = nc.declare_dram_parameter("bkk", [72, 2 * 9 * 128], BF16, isOutput=False)
    maskA_d = nc.declare_dram_parameter("maskA", [128, NSUB * 72], BF16, isOutput=False)
    cntT_d = nc.declare_dram_parameter("cntT", [128, NSUB], F32, isOutput=False)
    out_d = nc.declare_dram_parameter("out", [N, C], F32, isOutput=True)
    at_d = nc.dram_tensor("at_scratch", [80, N], BF16, kind="Internal")[:, :]

    with tile.TileContext(nc) as tc, ExitStack() as ctx:
        singles = ctx.enter_context(tc.tile_pool(name="singles", bufs=1))
        qkv_pool = ctx.enter_context(tc.tile_pool(name="qkv", bufs=1))

        identr = singles.tile([128, 128], F32R)
        nc.sync.dma_start(out=identr, in_=ident_d[:, :].bitcast(F32R))
        identb = singles.tile([128, 128], BF16)
        nc.sync.dma_start(out=identb, in_=identb_d[:, :])
        ones_s = singles.tile([128, 4], BF16)
        nc.gpsimd.dma_start(out=ones_s, in_=ones_s_d[:, :])
        maskA = singles.tile([128, NSUB, 72], BF16)
        nc.gpsimd.dma_start(
            out=maskA, in_=maskA_d[:, :].rearrange("p (s f) -> p s f", f=72))
        cntT = singles.tile([128, NSUB], F32)
        nc.gpsimd.dma_start(out=cntT, in_=cntT_d[:, :])

        # q/k/v in transposed [c, m] bf16 layout; k/v have zero borders of PAD
        qT = [qkv_pool.tile([128, N], BF16, name=f"qT{j}") for j in range(2)]
        kT = [qkv_pool.tile([128, N + 2 * PAD], BF16, name=f"kT{j}") for j in range(2)]
        vT = [qkv_pool.tile([128, N + 2 * PAD], BF16, name=f"vT{j}") for j in range(2)]
        for j in range(2):
            nc.gpsimd.memset(kT[j][:, 0:PAD], 0.0)
            nc.gpsimd.memset(kT[j][:, PAD + N:], 0.0)
            nc.gpsimd.memset(vT[j][:, 0:PAD], 0.0)
            nc.gpsimd.memset(vT[j][:, PAD + N:], 0.0)

        # normalized attention, [72 rows = h*9+kk, m]
        attn_pool = ctx.enter_context(tc.tile_pool(name="attn_sb", bufs=1))
        at72 = attn_pool.tile([72, N], BF16)

        # ---- P1: W^T tiles and x^T via PE transpose (f32r) ----
        xt_pool = tc.alloc_tile_pool(name="xt_pool", bufs=1)
        with tc.tile_pool(name="trans_sb", bufs=4) as tsb, \
             tc.tile_pool(name="trans_ps", bufs=2, space="PSUM") as tps:
            wlhsT = [singles.tile([128, 6, 128], F32R, name=f"wlhsT{j}") for j in range(2)]
            for ot in range(6):
                w_rows = tsb.tile([128, 256], F32R, name="w_rows")
                nc.sync.dma_start(out=w_rows, in_=w_d[ot * 128:(ot + 1) * 128, :].bitcast(F32R))
                for j in range(2):
                    wt_ps = tps.tile([128, 128], F32R, name="wt_ps")
                    nc.tensor.transpose(wt_ps, w_rows[:, j * 128:(j + 1) * 128], identr)
                    nc.scalar.copy(out=wlhsT[j][:, ot, :], in_=wt_ps)

            xT = [xt_pool.tile([128, N], F32R, name=f"xT{j}") for j in range(2)]
            xin = x_d[:, :].rearrange("(t p) c -> p t c", p=128).bitcast(F32R)
            for mb in range(8):
                x_rows = tsb.tile([128, 4, 256], F32R, name="x_rows")
                qeng = [nc.sync, nc.gpsimd, nc.scalar][mb % 3]
                qeng.dma_start(out=x_rows, in_=xin[:, mb * 4:(mb + 1) * 4, :])
                for t in range(4):
                    mt = mb * 4 + t
                    xt_ps = tps.tile([128, 256], F32R, name="xt_ps")
                    for j in range(2):
                        nc.tensor.transpose(
                            xt_ps[:, j * 128:(j + 1) * 128],
                            x_rows[:, t, j * 128:(j + 1) * 128], identr)
                    for j in range(2):
                        dst = xT[j][:, mt * 128:(mt + 1) * 128]
                        src = xt_ps[:, j * 128:(j + 1) * 128]
                        if (mt * 2 + j) % 4 != 3:
                            nc.vector.tensor_copy(out=dst, in_=src)
                        else:
                            nc.scalar.copy(out=dst, in_=src)

        # ---- P2: qkv projection (f32r) -> bf16 qT/kT/vT ----
        with tc.tile_pool(name="qkv_ps", bufs=4, space="PSUM") as qps:
            for ot in range(6):
                for ch in range(8):
                    acc = qps.tile([128, 512], F32, name="acc")
                    for j in range(2):
                        nc.tensor.matmul(
                            acc, wlhsT[j][:, ot, :],
                            xT[j][:, ch * 512:(ch + 1) * 512],
                            start=(j == 0), stop=(j == 1))
                    dst_j = ot % 2
                    if ot < 2:
                        dst = qT[dst_j][:, ch * 512:(ch + 1) * 512]
                    elif ot < 4:
                        dst = kT[dst_j][:, PAD + ch * 512:PAD + (ch + 1) * 512]
                    else:
                        dst = vT[dst_j][:, PAD + ch * 512:PAD + (ch + 1) * 512]
                    if (ot * 8 + ch) % 3 != 0:
                        nc.scalar.copy(out=dst, in_=acc)
                    else:
                        nc.vector.tensor_copy(out=dst, in_=acc)
        xt_pool.release()

        # ---- P3/P4, software-pipelined over m-chunks ----
        out_view = out_d[:, :].rearrange(
            "(s p) (j c) -> p s j c", p=128, j=2)

        with tc.tile_pool(name="prod_sb", bufs=1) as ttb, \
             tc.tile_pool(name="t2_sb", bufs=4) as t2b, \
             tc.tile_pool(name="bcd_sb", bufs=1) as bsb, \
             tc.tile_pool(name="sm_sb", bufs=4) as smb, \
             tc.tile_pool(name="sc_ps", bufs=2, space="PSUM") as sps, \
             tc.tile_pool(name="at_ps", bufs=2, space="PSUM") as aps, \
             tc.tile_pool(name="o_ps", bufs=1, space="PSUM") as ops, \
             tc.tile_pool(name="o_sb", bufs=2) as osb:

            def front(ch):
                """products -> scores -> softmax -> normalized at72 -> DRAM."""
                m0 = ch * MCH
                t_t = [[None] * K2 for _ in range(2)]
                for kk in range(K2):
                    dl = DELTAS[kk]
                    for j in range(2):
                        t = ttb.tile([128, MCH], BF16, name=f"pr{j}_{kk}")
                        if _prod_engine('qk', ch, j, kk) == 'pool':
                            nc.gpsimd.tensor_mul(
                                t, qT[j][:, m0:m0 + MCH],
                                kT[j][:, PAD + m0 + dl:PAD + m0 + MCH + dl])
                        else:
                            nc.vector.tensor_mul(
                                t, qT[j][:, m0:m0 + MCH],
                                kT[j][:, PAD + m0 + dl:PAD + m0 + MCH + dl])
                        t_t[j][kk] = t
                for g in range(SUBS // 4):
                    s_ps = sps.tile([128, 4, 72], F32, name="s_ps")
                    for sub4 in range(4):
                        sub = g * 4 + sub4
                        for kk in range(K2):
                            for j in range(2):
                                out_ap = s_ps.rearrange(
                                    "p s (h k) -> p s h k", k=9)[:, sub4, 4 * j:4 * j + 4, kk]
                                nc.tensor.matmul(
                                    out_ap,
                                    t_t[j][kk][:, sub * 128:sub * 128 + 128],
                                    ones_s, start=True, stop=True)
                    ms0 = ch * SUBS + g * 4
                    em0 = smb.tile([128, 4, 72], BF16, name="em0")
                    nc.scalar.activation(
                        em0, s_ps, mybir.ActivationFunctionType.Exp,
                        scale=float(SCALE))
                    em = smb.tile([128, 4, 72], BF16, name="em")
                    nc.vector.tensor_mul(em, em0, maskA[:, ms0:ms0 + 4, :])
                    den = smb.tile([128, 4, 8], F32, name="den")
                    nc.vector.reduce_sum(
                        den, em.rearrange("p s (h k) -> p s h k", k=9),
                        axis=mybir.AxisListType.X)
                    cb = cntT[:, ms0:ms0 + 4].unsqueeze(2).broadcast_to([128, 4, 8])
                    nc.vector.scalar_tensor_tensor(
                        out=den, in0=den, scalar=1.0, in1=cb,
                        op0=mybir.AluOpType.mult, op1=mybir.AluOpType.add)
                    rr = smb.tile([128, 4, 8], F32, name="rr")
                    nc.vector.reciprocal(rr, den)
                    emn = smb.tile([128, 4, 72], BF16, name="emn")
                    rbc = rr[:, :, :].unsqueeze(3).broadcast_to([128, 4, 8, 9])
                    nc.vector.scalar_tensor_tensor(
                        out=emn.rearrange("p s (h k) -> p s h k", k=9),
                        in0=em.rearrange("p s (h k) -> p s h k", k=9),
                        scalar=1.0, in1=rbc,
                        op0=mybir.AluOpType.mult, op1=mybir.AluOpType.mult)
                    at_ps = aps.tile([72, 4, 128], F32, name="at_ps")
                    for sub4 in range(4):
                        nc.tensor.matmul(
                            at_ps[:, sub4, :], emn[:, sub4, :], identb,
                            start=True, stop=True)
                    nc.scalar.copy(
                        out=at72[:, m0 + g * 512:m0 + (g + 1) * 512],
                        in_=at_ps.rearrange("p s q -> p (s q)"))
                nc.sync.dma_start(out=at_d[0:72, m0:m0 + MCH],
                                  in_=at72[:, m0:m0 + MCH])

            def back(ch):
                """DMA-broadcast attn, attn*v products, transpose-accumulate."""
                m0 = ch * MCH
                bc_t = [[None] * K2 for _ in range(2)]
                nd = 0
                for kk in range(K2):
                    for j in range(2):
                        bc = bsb.tile([128, MCH], BF16, name=f"bcd{j}_{kk}")
                        r0 = (4 * j) * 9 + kk
                        bap = at_d[r0:r0 + 28:9, m0:m0 + MCH]
                        bap = bap.unsqueeze(1).broadcast_to([4, 32, MCH])
                        qeng = [nc.sync, nc.gpsimd, nc.scalar][nd % 3]
                        nd += 1
                        qeng.dma_start(out=bc, in_=bap)
                        bc_t[j][kk] = (bc, None)
                o_sb = osb.tile([128, SUBS, 256], F32, name="o_sb")
                for half in range(2):
                    h0 = half * 512
                    o_gs = [ops.tile([128, 512], F32, name=f"o_g{sub4}")
                            for sub4 in range(4)]
                    for j in range(2):
                        for kk in range(K2):
                            dl = DELTAS[kk]
                            bch = bc_t[j][kk][0][:, h0:h0 + 512]
                            t2 = t2b.tile([128, 512], BF16, name=f"t2_{(j * K2 + kk) % 3}")
                            vsl = vT[j][:, PAD + m0 + h0 + dl:PAD + m0 + h0 + 512 + dl]
                            if _prod_engine('av', ch, j, kk) == 'pool':
                                nc.gpsimd.tensor_mul(t2, bch, vsl)
                            else:
                                nc.vector.tensor_mul(t2, bch, vsl)
                            for sub4 in range(4):
                                nc.tensor.matmul(
                                    o_gs[sub4][:, j * 128:(j + 1) * 128],
                                    t2[:, sub4 * 128:(sub4 + 1) * 128], identb,
                                    start=(kk == 0), stop=(kk == K2 - 1))
                        # evacuate this j's closed groups while the other j runs
                        for sub4 in range(4):
                            sub = half * 4 + sub4
                            dst = o_sb[:, sub, j * 128:(j + 1) * 128]
                            src = o_gs[sub4][:, j * 128:(j + 1) * 128]
                            nc.scalar.copy(out=dst, in_=src)
                    s0 = ch * SUBS + half * 4
                    nc.sync.dma_start(
                        out=out_view[:, s0:s0 + 4, :, :],
                        in_=o_sb[:, half * 4:half * 4 + 4, :].rearrange(
                            "p s (j c) -> p s j c", j=2))

            front(0)
            for ch in range(1, NCH):
                front(ch)
                back(ch - 1)
            back(NCH - 1)
    nc.compile()
    return nc


_NC_CACHE = None


def kernel(x: np.ndarray, W_qkv: np.ndarray) -> np.ndarray:
    global _NC_CACHE
    if _NC_CACHE is None:
        _NC_CACHE = build_nc()
    nc = _NC_CACHE

    x = np.ascontiguousarray(x, dtype=np.float32)
    W_qkv = np.ascontiguousarray(W_qkv, dtype=np.float32)
    ident, identb, ones_s, bkk, maskA, cntT = _host_consts()
    consts = {
        "w": W_qkv, "ident": ident, "identb": identb, "ones_s": ones_s,
        "bkk": bkk, "maskA": maskA, "cntT": cntT,
    }
    in_maps = [
        {"x": x[b].reshape(N, C).copy(), **consts} for b in range(B)
    ]
    res = run_bass_kernel_spmd(nc, in_maps, list(range(B)))
    out = np.stack([res.results[b]["out"].reshape(H, W, C) for b in range(B)])
    return out


if __name__ == "__main__":
    rng = np.random.default_rng(0)
    x = rng.standard_normal((B, H, W, C), dtype=np.float32)
    wq = (rng.standard_normal((3 * C, C), dtype=np.float32) * 0.02).astype(np.float32)
    out = kernel(x, wq)
    print("out", out.shape, out.dtype, float(np.abs(out).mean()))



# revision 3
# speedup vs baseline: 1.2376x; 1.2376x over previous
"""Trainium2 Bass kernel v4 for dilated local attention (nn_DilateAttention).

Problem: x [8, 64, 64, 256] f32, W_qkv [768, 256] f32.
  qkv = x @ W_qkv.T; per pixel, per head (8 heads x 32 dim): attention over
  the 9 dilated (3x3, dilation 3) spatial neighbors with zero padding.

v4 strategy (1 image per core, j-merged [c(128), 2, m] on-chip layout):
  - host pre-casts x/W to bf16; x^T/W^T via DMA XBAR transposes.
  - PE warm-up chain beats the tensor-engine p-state ramp.
  - qkv projection on PE; both c-halves of q/k/v live in single
    [128, 2, span] tiles so every product / broadcast / evacuation handles
    both halves in ONE instruction (halves instruction count).
  - scores: q*k products on DVE (bf16 2x) with a Pool minority; per-head
    sums via tiny product-as-stationary matmuls; the column-validity mask
    is pre-loaded into the score PSUM bank by a constant matmul, so exp()
    masks for free; denominator gets a constant count-correction.
  - softmax: exp on Act, reduce/recip/normalize on DVE/Pool smalls.
  - normalized attention transposed to [72, m] by PE, evacuated by Act,
    bounced through a DRAM scratch, and broadcast to [c, 2, m] tiles by
    partition-replicating DMAs spread over SP/Act/Pool queues.
  - AV: in-place products (bc *= v) interleaved with PE transpose-
    accumulate; rows-layout f32 PSUM evacuated once per 512-px group and
    DMA'd to DRAM.  GPSIMD never touches PSUM (hardware constraint).
"""

import sys

sys.path.insert(0, "/opt/trn_rl_repo")

import numpy as np
import ml_dtypes
from contextlib import ExitStack

import concourse.bass as bass
import concourse.bacc as bacc
import concourse.tile as tile
from concourse import mybir
from concourse.bass_utils import run_bass_kernel_spmd

B, H, W, C = 8, 64, 64, 256
NH, DPH, K2 = 8, 32, 9
N = H * W          # 4096 pixels
PAD = 256          # zero border on each side of k/v (covers |delta| <= 195)
PADW = N + 2 * PAD
MCH = 1024         # pixels per m-chunk
NCH = N // MCH     # 4 chunks
SUBS = MCH // 128  # 8 m-subs per chunk
SCALE = DPH ** -0.5
F32 = mybir.dt.float32
BF16 = mybir.dt.bfloat16
NPBF16 = ml_dtypes.bfloat16
MASKVAL = -16384.0

DELTAS = [64 * (3 * i - 3) + (3 * j - 3) for i in range(3) for j in range(3)]

_CONST_NAMES = ["identb", "ones_s", "m72", "rmask", "cnt1"]

# product engine per kk ('d' DVE / 'p' Pool); Pool is ~1.6x slower per
# element in the CoreSim model, 3/9 keeps the two in balance with their
# other duties.
_QK_POOL = {1, 4, 7}
_AV_POOL = {1, 4, 7}


def _host_consts():
    identb = np.eye(128, dtype=NPBF16)
    # score reduce (moving operand): ones_s[p, hh] = 1 iff p//32 == hh
    ones_s = np.zeros((128, 4), NPBF16)
    for p in range(128):
        ones_s[p, p // 32] = 1.0
    # mask bias, [f=(h*9+kk), m-local(128)]: 0 if column-valid else MASKVAL.
    m72 = np.zeros((72, 128), NPBF16)
    cnt1 = np.zeros((128, 1), np.float32)
    for p in range(128):
        col = p % 64
        for kk in range(K2):
            dc = 3 * (kk % 3) - 3
            if not (0 <= col + dc < 64):
                for h in range(8):
                    m72[h * 9 + kk, p] = MASKVAL
        cnt1[p, 0] = sum(
            1 for kk in range(K2) if not (0 <= (p % 64) + 3 * (kk % 3) - 3 < 64))
    # rmask[p, s4*72 + f] = delta(p, f) for s4 < 4; cols 288..511 zero.
    rmask = np.zeros((72, 512), NPBF16)
    for s4 in range(4):
        for f in range(72):
            rmask[f, s4 * 72 + f] = 1.0
    return identb, ones_s, m72, rmask, cnt1


def build_nc() -> bass.Bass:
    nc = bacc.Bacc()
    x_d = nc.declare_dram_parameter("x", [N, C], BF16, isOutput=False)
    w_d = nc.declare_dram_parameter("w", [3 * C, C], BF16, isOutput=False)
    identb_d = nc.declare_dram_parameter("identb", [128, 128], BF16, isOutput=False)
    ones_s_d = nc.declare_dram_parameter("ones_s", [128, 4], BF16, isOutput=False)
    m72_d = nc.declare_dram_parameter("m72", [72, 128], BF16, isOutput=False)
    rmask_d = nc.declare_dram_parameter("rmask", [72, 512], BF16, isOutput=False)
    cnt1_d = nc.declare_dram_parameter("cnt1", [128, 1], F32, isOutput=False)
    out_d = nc.declare_dram_parameter("out", [N, C], F32, isOutput=True)
    at_d = nc.dram_tensor("at_scratch", [36, 2 * N], BF16, kind="Internal")[:, :]

    with tile.TileContext(nc) as tc, ExitStack() as ctx:
        singles = ctx.enter_context(tc.tile_pool(name="singles", bufs=1))
        qkv_pool = ctx.enter_context(tc.tile_pool(name="qkv", bufs=1))

        # q/k/v, both c-halves per tile: [c(128), j(2), m]; k/v zero borders
        qT = qkv_pool.tile([128, 2, N], BF16, name="qT")
        kT = qkv_pool.tile([128, 2, PADW], BF16, name="kT")
        vT = qkv_pool.tile([128, 2, PADW], BF16, name="vT")
        nc.gpsimd.memset(kT[:, :, 0:PAD], 0.0)
        nc.gpsimd.memset(kT[:, :, PAD + N:], 0.0)
        nc.gpsimd.memset(vT[:, :, 0:PAD], 0.0)
        nc.gpsimd.memset(vT[:, :, PAD + N:], 0.0)

        # consts: m72/rmask on sync (feed the scheduler-hoisted PE mask
        # matmuls), the rest on scalar.
        m72 = singles.tile([72, 128], BF16)
        nc.sync.dma_start(out=m72, in_=m72_d[:, :])
        rmask = singles.tile([72, 512], BF16)
        nc.sync.dma_start(out=rmask, in_=rmask_d[:, :])
        identb = singles.tile([128, 128], BF16)
        nc.scalar.dma_start(out=identb, in_=identb_d[:, :])
        ones_s = singles.tile([128, 4], BF16)
        nc.scalar.dma_start(out=ones_s, in_=ones_s_d[:, :])
        cnt1 = singles.tile([128, 1], F32)
        nc.scalar.dma_start(out=cnt1, in_=cnt1_d[:, :])

        attn_pool = ctx.enter_context(tc.tile_pool(name="attn_sb", bufs=1))
        at72 = attn_pool.tile([72, N], BF16)

        sm_pool = ctx.enter_context(tc.tile_pool(name="sm_sb", bufs=2))

        wT = [singles.tile([128, 6 * 128], BF16, name=f"wT{j}") for j in range(2)]

        out_view = out_d[:, :].rearrange("(s p) (j c) -> p s j c", p=128, j=2)

        with tc.tile_pool(name="sc_ps", bufs=2, space="PSUM") as sps, \
             tc.tile_pool(name="at_ps", bufs=2, space="PSUM") as aps, \
             tc.tile_pool(name="prod_sb", bufs=1) as ttb, \
             tc.tile_pool(name="bcd_sb", bufs=2) as bsb:

            # ---- P0: PE warm-up against the p-state ramp
            warm = ttb.tile([128, 512], BF16, name="warm")
            nc.gpsimd.memset(warm, 0.0)
            for wi in range(7):
                wp = aps.tile([72, 4, 128], F32, name="at_ps")
                nc.tensor.matmul(
                    wp.rearrange("p s q -> p (s q)"), warm[:, 0:72],
                    warm, start=True, stop=True)

            # ---- P1: W^T via single-shot XBAR transposes (hardware-safe
            # pattern); x^T on the warm PE (multi-piece XBAR transposes into
            # one tile corrupt on hardware).
            xt_pool = tc.alloc_tile_pool(name="xt_pool", bufs=1)
            xT = [xt_pool.tile([128, N], BF16, name=f"xT{j}") for j in range(2)]

            for j in range(2):
                qeng = [nc.sync, nc.scalar][j]
                qeng.dma_start(out=wT[j],
                               in_=w_d[:, 128 * j:128 * (j + 1)],
                               transpose=True)
            xin = x_d[:, :].rearrange("(t p) c -> p t c", p=128)
            xr_pool = tc.alloc_tile_pool(name="xr_pool", bufs=2)
            with tc.tile_pool(name="xtr_ps", bufs=2, space="PSUM") as tps:
                for q4 in range(4):
                    x_rows = xr_pool.tile([128, 8, 256], BF16, name="x_rows")
                    qeng = [nc.sync, nc.scalar][q4 % 2]
                    qeng.dma_start(out=x_rows,
                                   in_=xin[:, q4 * 8:(q4 + 1) * 8, :])
                    for j in range(2):
                        xt_ps = tps.tile([128, 8, 128], BF16, name="xt_ps")
                        for t in range(8):
                            nc.tensor.transpose(
                                xt_ps[:, t, :],
                                x_rows[:, t, 128 * j:128 * (j + 1)],
                                identb)
                        dst = xT[j][:, q4 * 1024:(q4 + 1) * 1024]
                        if (j + q4) % 2 == 0:
                            nc.vector.tensor_copy(
                                out=dst, in_=xt_ps.rearrange("p t q -> p (t q)"))
                        else:
                            nc.scalar.copy(
                                out=dst, in_=xt_ps.rearrange("p t q -> p (t q)"))
            xr_pool.release()

            def project(units, evac_rr):
                """units: (fam, mch); fam 0=q 1=k 2=v; one [128,2,512] acc."""
                for i, (fam, mch) in enumerate(units):
                    acc = project.qps.tile([128, 2, 512], F32, name="acc")
                    for dj in range(2):
                        for j in range(2):
                            nc.tensor.matmul(
                                acc[:, dj, :],
                                wT[j][:, (2 * fam + dj) * 128:(2 * fam + dj + 1) * 128],
                                xT[j][:, mch * 512:(mch + 1) * 512],
                                start=(j == 0), stop=(j == 1))
                    base = 0 if fam == 0 else PAD
                    dst = [qT, kT, vT][fam][:, :, base + mch * 512:base + (mch + 1) * 512]
                    e = evac_rr[i % len(evac_rr)]
                    if e == 'a':
                        nc.scalar.copy(out=dst, in_=acc)
                    else:
                        nc.vector.tensor_copy(out=dst, in_=acc)

            def front_products(ch, kks, t_t):
                m0 = ch * MCH
                for kk in kks:
                    dl = DELTAS[kk]
                    t = ttb.tile([128, 2, MCH], BF16, name=f"pr{kk}")
                    eng = nc.gpsimd if kk in _QK_POOL else nc.vector
                    eng.tensor_mul(
                        t, qT[:, :, m0:m0 + MCH],
                        kT[:, :, PAD + m0 + dl:PAD + m0 + MCH + dl])
                    t_t[kk] = t

            def front(ch, t_t=None, kks=None):
                """products -> masked scores -> softmax -> at72 -> at_d."""
                m0 = ch * MCH
                if t_t is None:
                    t_t = [None] * K2
                front_products(ch, kks if kks is not None else range(K2), t_t)
                for g in range(SUBS // 4):
                    s_bank = sps.tile([128, 512], F32, name="s_ps")
                    s_ps = s_bank[:, 0:288].rearrange("p (s f) -> p s f", f=72)
                    nc.tensor.matmul(
                        s_bank, m72, rmask, start=True, stop=False)
                    for sub4 in range(4):
                        sub = g * 4 + sub4
                        for kk in range(K2):
                            for j in range(2):
                                last = (sub4 == 3 and kk == K2 - 1 and j == 1)
                                out_ap = s_ps.rearrange(
                                    "p s (h k) -> p s h k", k=9)[:, sub4, 4 * j:4 * j + 4, kk]
                                nc.tensor.matmul(
                                    out_ap,
                                    t_t[kk][:, j, sub * 128:sub * 128 + 128],
                                    ones_s, start=False, stop=last)
                    em = sm_pool.tile([128, 4, 72], BF16, name="em")
                    nc.scalar.activation(
                        em, s_ps, mybir.ActivationFunctionType.Exp,
                        scale=float(SCALE))
                    den = sm_pool.tile([128, 4, 8], F32, name="den")
                    nc.vector.reduce_sum(
                        den, em.rearrange("p s (h k) -> p s h k", k=9),
                        axis=mybir.AxisListType.X)
                    nc.gpsimd.tensor_scalar_add(den, den, cnt1[:, 0:1])
                    rr = sm_pool.tile([128, 4, 8], F32, name="rr")
                    nc.vector.reciprocal(rr, den)
                    rrb = sm_pool.tile([128, 4, 8], BF16, name="rrb")
                    nc.gpsimd.tensor_copy(out=rrb, in_=rr)
                    emn = sm_pool.tile([128, 4, 72], BF16, name="emn")
                    nc.vector.tensor_mul(
                        emn.rearrange("p s (h k) -> p s h k", k=9),
                        em.rearrange("p s (h k) -> p s h k", k=9),
                        rrb.unsqueeze(3).broadcast_to([128, 4, 8, 9]))
                    at_ps = aps.tile([72, 4, 128], F32, name="at_ps")
                    for sub4 in range(4):
                        nc.tensor.matmul(
                            at_ps[:, sub4, :], emn[:, sub4, :], identb,
                            start=True, stop=True)
                    nc.scalar.copy(
                        out=at72[:, m0 + g * 512:m0 + (g + 1) * 512],
                        in_=at_ps.rearrange("p s q -> p (s q)"))
                    # store this group's half of at_d right away so back()
                    # can begin its broadcasts before the whole chunk is done
                    # layout: at_d[hh*9+kk, ch*2048 + g*1024 + jj*512 + m]
                    for jj in range(2):
                        qeng = [nc.sync, nc.scalar][(g + jj) % 2]
                        qeng.dma_start(
                            out=at_d[0:36, 2 * m0 + g * MCH + jj * 512:
                                     2 * m0 + g * MCH + (jj + 1) * 512],
                            in_=at72[36 * jj:36 * (jj + 1),
                                     m0 + g * 512:m0 + (g + 1) * 512])


            def back(ch, m0=None, width=None):
                """broadcast attn, in-place attn*v, transpose-accum, DMA."""
                if m0 is None:
                    m0 = ch * MCH
                if width is None:
                    width = MCH
                ngrp = width // 512
                last = (ch == NCH - 1)
                bc_t = [None] * K2
                ch0 = m0 // MCH
                gof = (m0 % MCH) // 512
                for kk in range(K2):
                    bc = bsb.tile([128, 2 * width], BF16, name=f"bcd{kk}")
                    base = 2 * ch0 * MCH + gof * MCH
                    src = at_d[kk:kk + 28:9, base:base + 2 * width]
                    src = src.unsqueeze(1).broadcast_to([4, 32, 2 * width])
                    qeng = [nc.sync, nc.gpsimd, nc.sync][kk % 3]
                    qeng.dma_start(out=bc, in_=src)
                    bc_t[kk] = bc
                o_ps = {g: back.ops.tile([128, 2, 4, 128], F32, name=f"o_ps{g}")
                        for g in range(ngrp)}
                # explicitly zero each bank with a full-footprint start=True
                # matmul (hardware zeroes per instruction footprint, not per
                # bank, so the first write of every byte must be a start)
                for g in range(ngrp):
                    for j in range(2):
                        nc.tensor.matmul(
                            o_ps[g][:, j, :, :].rearrange("p s q -> p (s q)"),
                            warm[:, 0:128], warm, start=True, stop=False)
                for kk in range(K2):
                    dl = DELTAS[kk]
                    eng = nc.gpsimd if kk in _AV_POOL else nc.vector
                    vsl = vT[:, :, PAD + m0 + dl:PAD + m0 + width + dl]
                    if width == MCH:
                        # bc layout [c, g, jj, 512]
                        bcv = bc_t[kk].rearrange("p (g jj m) -> p g jj m",
                                                 g=2, jj=2)
                        eng.tensor_mul(
                            bcv, bcv,
                            vsl.rearrange("p jj (g m) -> p g jj m", g=2))
                    else:
                        # bc layout [c, jj, 512]
                        bcv = bc_t[kk].rearrange("p (jj m) -> p jj m", jj=2)
                        eng.tensor_mul(bcv, bcv, vsl)
                    for j in range(2):
                        for g in range(ngrp):
                            for sub4 in range(4):
                                if width == MCH:
                                    lhs = bc_t[kk].rearrange(
                                        "p (g jj m) -> p g jj m", g=2, jj=2)[
                                        :, g, j, sub4 * 128:(sub4 + 1) * 128]
                                else:
                                    lhs = bc_t[kk].rearrange(
                                        "p (jj m) -> p jj m", jj=2)[
                                        :, j, sub4 * 128:(sub4 + 1) * 128]
                                nc.tensor.matmul(
                                    o_ps[g][:, j, sub4, :], lhs, identb,
                                    start=False,
                                    stop=(kk == K2 - 1 and sub4 == 3))
                for g in range(ngrp):
                    o_sb = back.osb.tile([128, 2, 4, 128], F32, name="o_sb")
                    if last and g % 2 == 1:
                        nc.vector.tensor_copy(out=o_sb, in_=o_ps[g])
                    else:
                        nc.scalar.copy(out=o_sb, in_=o_ps[g])
                    s0 = m0 // 128 + g * 4
                    for j in range(2):
                        qeng = [nc.sync, nc.scalar, nc.gpsimd][(g * 2 + j) % 3] \
                            if last else [nc.sync, nc.scalar][(g + j) % 2]
                        qeng.dma_start(
                            out=out_view[:, s0:s0 + 4, j, :],
                            in_=o_sb[:, j, :, :])

            with tc.tile_pool(name="qkv_ps", bufs=2, space="PSUM") as qps:
                project.qps = qps
                rr = ['a']
                project([(fam, mch) for mch in range(2) for fam in range(2)], rr)
                t0 = [None] * K2
                front_products(0, range(3), t0)
                project([(fam, 2) for fam in range(2)], rr)
                front(0, t_t=t0, kks=range(3, K2))
                project([(fam, mch) for mch in range(3, 8) for fam in range(2)], rr)
                project([(2, mch) for mch in range(8)], rr)
            xt_pool.release()
            with tc.tile_pool(name="o_ps", bufs=1, space="PSUM") as ops, \
                 tc.tile_pool(name="o_sb", bufs=2) as osb:
                back.ops = ops
                back.osb = osb
                for ch in range(1, NCH):
                    front(ch)
                    back(ch - 1)
                back(NCH - 1, m0=(NCH - 1) * MCH, width=512)
                back(NCH - 1, m0=(NCH - 1) * MCH + 512, width=512)
    nc.compile()
    return nc


_NC_CACHE = None


def make_in_map(x_core: np.ndarray, W_qkv: np.ndarray) -> dict:
    """Per-core input dict. x_core: [N, C] f32, W_qkv: [768, 256] f32."""
    consts = dict(zip(_CONST_NAMES, _host_consts()))
    return {
        "x": np.ascontiguousarray(x_core.reshape(N, C)).astype(NPBF16),
        "w": np.ascontiguousarray(W_qkv).astype(NPBF16),
        **consts,
    }


def kernel(x: np.ndarray, W_qkv: np.ndarray) -> np.ndarray:
    global _NC_CACHE
    if _NC_CACHE is None:
        _NC_CACHE = build_nc()
    nc = _NC_CACHE

    x = np.ascontiguousarray(x, dtype=np.float32)
    W_qkv = np.ascontiguousarray(W_qkv, dtype=np.float32)
    in_maps = [make_in_map(x[b], W_qkv) for b in range(B)]
    res = run_bass_kernel_spmd(nc, in_maps, list(range(B)))
    out = np.stack([res.results[b]["out"].reshape(H, W, C) for b in range(B)])
    return out


if __name__ == "__main__":
    rng = np.random.default_rng(0)
    x = rng.standard_normal((B, H, W, C), dtype=np.float32)
    wq = (rng.standard_normal((3 * C, C), dtype=np.float32) * 0.02).astype(np.float32)
    out = kernel(x, wq)
    print("out", out.shape, out.dtype, float(np.abs(out).mean()))


# revision 4
# speedup vs baseline: 1.2448x; 1.0058x over previous
"""Trainium2 Bass kernel v4 for dilated local attention (nn_DilateAttention).

Problem: x [8, 64, 64, 256] f32, W_qkv [768, 256] f32.
  qkv = x @ W_qkv.T; per pixel, per head (8 heads x 32 dim): attention over
  the 9 dilated (3x3, dilation 3) spatial neighbors with zero padding.

v4 strategy (1 image per core, j-merged [c(128), 2, m] on-chip layout):
  - host pre-casts x/W to bf16; x^T/W^T via DMA XBAR transposes.
  - PE warm-up chain beats the tensor-engine p-state ramp.
  - qkv projection on PE; both c-halves of q/k/v live in single
    [128, 2, span] tiles so every product / broadcast / evacuation handles
    both halves in ONE instruction (halves instruction count).
  - scores: q*k products on DVE (bf16 2x) with a Pool minority; per-head
    sums via tiny product-as-stationary matmuls; the column-validity mask
    is pre-loaded into the score PSUM bank by a constant matmul, so exp()
    masks for free; denominator gets a constant count-correction.
  - softmax: exp on Act, reduce/recip/normalize on DVE/Pool smalls.
  - normalized attention transposed to [72, m] by PE, evacuated by Act,
    bounced through a DRAM scratch, and broadcast to [c, 2, m] tiles by
    partition-replicating DMAs spread over SP/Act/Pool queues.
  - AV: in-place products (bc *= v) interleaved with PE transpose-
    accumulate; rows-layout f32 PSUM evacuated once per 512-px group and
    DMA'd to DRAM.  GPSIMD never touches PSUM (hardware constraint).
"""

import sys

sys.path.insert(0, "/opt/trn_rl_repo")

import numpy as np
import ml_dtypes
from contextlib import ExitStack

import concourse.bass as bass
import concourse.bacc as bacc
import concourse.tile as tile
from concourse import mybir
from concourse.bass_utils import run_bass_kernel_spmd

B, H, W, C = 8, 64, 64, 256
NH, DPH, K2 = 8, 32, 9
N = H * W          # 4096 pixels
PAD = 256          # zero border on each side of k/v (covers |delta| <= 195)
PADW = N + 2 * PAD
MCH = 1024         # pixels per m-chunk
NCH = N // MCH     # 4 chunks
SUBS = MCH // 128  # 8 m-subs per chunk
SCALE = DPH ** -0.5
F32 = mybir.dt.float32
BF16 = mybir.dt.bfloat16
NPBF16 = ml_dtypes.bfloat16
MASKVAL = -16384.0

DELTAS = [64 * (3 * i - 3) + (3 * j - 3) for i in range(3) for j in range(3)]

_CONST_NAMES = ["identb", "ones_s", "m72", "rmask", "cnt1"]

# product engine per kk ('d' DVE / 'p' Pool); Pool is ~1.6x slower per
# element in the CoreSim model, 3/9 keeps the two in balance with their
# other duties.
_QK_POOL = {1, 4, 7}
_AV_POOL = {1, 4}


def _host_consts():
    identb = np.eye(128, dtype=NPBF16)
    # score reduce (moving operand): ones_s[p, hh] = 1 iff p//32 == hh
    ones_s = np.zeros((128, 4), NPBF16)
    for p in range(128):
        ones_s[p, p // 32] = 1.0
    # mask bias, [f=(h*9+kk), m-local(128)]: 0 if column-valid else MASKVAL.
    m72 = np.zeros((72, 128), NPBF16)
    cnt1 = np.zeros((128, 1), np.float32)
    for p in range(128):
        col = p % 64
        for kk in range(K2):
            dc = 3 * (kk % 3) - 3
            if not (0 <= col + dc < 64):
                for h in range(8):
                    m72[h * 9 + kk, p] = MASKVAL
        cnt1[p, 0] = sum(
            1 for kk in range(K2) if not (0 <= (p % 64) + 3 * (kk % 3) - 3 < 64))
    # rmask[p, s4*72 + f] = delta(p, f) for s4 < 4; cols 288..511 zero.
    rmask = np.zeros((72, 512), NPBF16)
    for s4 in range(4):
        for f in range(72):
            rmask[f, s4 * 72 + f] = 1.0
    return identb, ones_s, m72, rmask, cnt1


def build_nc() -> bass.Bass:
    nc = bacc.Bacc()
    x_d = nc.declare_dram_parameter("x", [N, C], BF16, isOutput=False)
    w_d = nc.declare_dram_parameter("w", [3 * C, C], BF16, isOutput=False)
    identb_d = nc.declare_dram_parameter("identb", [128, 128], BF16, isOutput=False)
    ones_s_d = nc.declare_dram_parameter("ones_s", [128, 4], BF16, isOutput=False)
    m72_d = nc.declare_dram_parameter("m72", [72, 128], BF16, isOutput=False)
    rmask_d = nc.declare_dram_parameter("rmask", [72, 512], BF16, isOutput=False)
    cnt1_d = nc.declare_dram_parameter("cnt1", [128, 1], F32, isOutput=False)
    out_d = nc.declare_dram_parameter("out", [N, C], F32, isOutput=True)
    at_d = nc.dram_tensor("at_scratch", [36, 2 * N], BF16, kind="Internal")[:, :]

    with tile.TileContext(nc) as tc, ExitStack() as ctx:
        singles = ctx.enter_context(tc.tile_pool(name="singles", bufs=1))
        qkv_pool = ctx.enter_context(tc.tile_pool(name="qkv", bufs=1))

        # q/k/v, both c-halves per tile: [c(128), j(2), m]; k/v zero borders
        qT = qkv_pool.tile([128, 2, N], BF16, name="qT")
        kT = qkv_pool.tile([128, 2, PADW], BF16, name="kT")
        vT = qkv_pool.tile([128, 2, PADW], BF16, name="vT")
        nc.gpsimd.memset(kT[:, :, 0:PAD], 0.0)
        nc.gpsimd.memset(kT[:, :, PAD + N:], 0.0)
        nc.gpsimd.memset(vT[:, :, 0:PAD], 0.0)
        nc.gpsimd.memset(vT[:, :, PAD + N:], 0.0)

        # consts: m72/rmask on sync (feed the scheduler-hoisted PE mask
        # matmuls), the rest on scalar.
        m72 = singles.tile([72, 128], BF16)
        nc.sync.dma_start(out=m72, in_=m72_d[:, :])
        rmask = singles.tile([72, 512], BF16)
        nc.sync.dma_start(out=rmask, in_=rmask_d[:, :])
        identb = singles.tile([128, 128], BF16)
        nc.scalar.dma_start(out=identb, in_=identb_d[:, :])
        ones_s = singles.tile([128, 4], BF16)
        nc.scalar.dma_start(out=ones_s, in_=ones_s_d[:, :])
        cnt1 = singles.tile([128, 1], F32)
        nc.scalar.dma_start(out=cnt1, in_=cnt1_d[:, :])

        attn_pool = ctx.enter_context(tc.tile_pool(name="attn_sb", bufs=1))
        at72 = attn_pool.tile([72, N], BF16)

        sm_pool = ctx.enter_context(tc.tile_pool(name="sm_sb", bufs=2))

        wT = [singles.tile([128, 6 * 128], BF16, name=f"wT{j}") for j in range(2)]

        out_view = out_d[:, :].rearrange("(s p) (j c) -> p s j c", p=128, j=2)

        with tc.tile_pool(name="sc_ps", bufs=2, space="PSUM") as sps, \
             tc.tile_pool(name="at_ps", bufs=2, space="PSUM") as aps, \
             tc.tile_pool(name="prod_sb", bufs=1) as ttb, \
             tc.tile_pool(name="bcd_sb", bufs=2) as bsb:

            # ---- P0: PE warm-up against the p-state ramp
            warm = ttb.tile([128, 512], BF16, name="warm")
            nc.gpsimd.memset(warm, 0.0)
            for wi in range(7):
                wp = aps.tile([72, 4, 128], F32, name="at_ps")
                nc.tensor.matmul(
                    wp.rearrange("p s q -> p (s q)"), warm[:, 0:72],
                    warm, start=True, stop=True)

            # ---- P1: W^T via single-shot XBAR transposes (hardware-safe
            # pattern); x^T on the warm PE (multi-piece XBAR transposes into
            # one tile corrupt on hardware).
            xt_pool = tc.alloc_tile_pool(name="xt_pool", bufs=1)
            xT = [xt_pool.tile([128, N], BF16, name=f"xT{j}") for j in range(2)]

            for j in range(2):
                qeng = [nc.sync, nc.scalar][j]
                qeng.dma_start(out=wT[j],
                               in_=w_d[:, 128 * j:128 * (j + 1)],
                               transpose=True)
            xin = x_d[:, :].rearrange("(t p) c -> p t c", p=128)
            xr_pool = tc.alloc_tile_pool(name="xr_pool", bufs=2)
            with tc.tile_pool(name="xtr_ps", bufs=2, space="PSUM") as tps:
                for q4 in range(4):
                    x_rows = xr_pool.tile([128, 8, 256], BF16, name="x_rows")
                    qeng = [nc.sync, nc.scalar][q4 % 2]
                    qeng.dma_start(out=x_rows,
                                   in_=xin[:, q4 * 8:(q4 + 1) * 8, :])
                    for j in range(2):
                        xt_ps = tps.tile([128, 8, 128], BF16, name="xt_ps")
                        for t in range(8):
                            nc.tensor.transpose(
                                xt_ps[:, t, :],
                                x_rows[:, t, 128 * j:128 * (j + 1)],
                                identb)
                        dst = xT[j][:, q4 * 1024:(q4 + 1) * 1024]
                        if (j + q4) % 2 == 0:
                            nc.vector.tensor_copy(
                                out=dst, in_=xt_ps.rearrange("p t q -> p (t q)"))
                        else:
                            nc.scalar.copy(
                                out=dst, in_=xt_ps.rearrange("p t q -> p (t q)"))
            xr_pool.release()

            def project(units, evac_rr):
                """units: (fam, mch); fam 0=q 1=k 2=v; one [128,2,512] acc."""
                for i, (fam, mch) in enumerate(units):
                    acc = project.qps.tile([128, 2, 512], F32, name="acc")
                    for dj in range(2):
                        for j in range(2):
                            nc.tensor.matmul(
                                acc[:, dj, :],
                                wT[j][:, (2 * fam + dj) * 128:(2 * fam + dj + 1) * 128],
                                xT[j][:, mch * 512:(mch + 1) * 512],
                                start=(j == 0), stop=(j == 1))
                    base = 0 if fam == 0 else PAD
                    dst = [qT, kT, vT][fam][:, :, base + mch * 512:base + (mch + 1) * 512]
                    e = evac_rr[i % len(evac_rr)]
                    if e == 'a':
                        nc.scalar.copy(out=dst, in_=acc)
                    else:
                        nc.vector.tensor_copy(out=dst, in_=acc)

            def front_products(ch, kks, t_t):
                m0 = ch * MCH
                for kk in kks:
                    dl = DELTAS[kk]
                    t = ttb.tile([128, 2, MCH], BF16, name=f"pr{kk}")
                    eng = nc.gpsimd if kk in _QK_POOL else nc.vector
                    eng.tensor_mul(
                        t, qT[:, :, m0:m0 + MCH],
                        kT[:, :, PAD + m0 + dl:PAD + m0 + MCH + dl])
                    t_t[kk] = t

            def front(ch, t_t=None, kks=None):
                """products -> masked scores -> softmax -> at72 -> at_d."""
                m0 = ch * MCH
                if t_t is None:
                    t_t = [None] * K2
                front_products(ch, kks if kks is not None else range(K2), t_t)
                for g in range(SUBS // 4):
                    s_bank = sps.tile([128, 512], F32, name="s_ps")
                    s_ps = s_bank[:, 0:288].rearrange("p (s f) -> p s f", f=72)
                    nc.tensor.matmul(
                        s_bank, m72, rmask, start=True, stop=False)
                    for sub4 in range(4):
                        sub = g * 4 + sub4
                        for kk in range(K2):
                            for j in range(2):
                                last = (sub4 == 3 and kk == K2 - 1 and j == 1)
                                out_ap = s_ps.rearrange(
                                    "p s (h k) -> p s h k", k=9)[:, sub4, 4 * j:4 * j + 4, kk]
                                nc.tensor.matmul(
                                    out_ap,
                                    t_t[kk][:, j, sub * 128:sub * 128 + 128],
                                    ones_s, start=False, stop=last)
                    em = sm_pool.tile([128, 4, 72], BF16, name="em")
                    nc.scalar.activation(
                        em, s_ps, mybir.ActivationFunctionType.Exp,
                        scale=float(SCALE))
                    den = sm_pool.tile([128, 4, 8], F32, name="den")
                    nc.vector.reduce_sum(
                        den, em.rearrange("p s (h k) -> p s h k", k=9),
                        axis=mybir.AxisListType.X)
                    nc.gpsimd.tensor_scalar_add(den, den, cnt1[:, 0:1])
                    rr = sm_pool.tile([128, 4, 8], F32, name="rr")
                    nc.vector.reciprocal(rr, den)
                    rrb = sm_pool.tile([128, 4, 8], BF16, name="rrb")
                    nc.gpsimd.tensor_copy(out=rrb, in_=rr)
                    emn = sm_pool.tile([128, 4, 72], BF16, name="emn")
                    nc.vector.tensor_mul(
                        emn.rearrange("p s (h k) -> p s h k", k=9),
                        em.rearrange("p s (h k) -> p s h k", k=9),
                        rrb.unsqueeze(3).broadcast_to([128, 4, 8, 9]))
                    at_ps = aps.tile([72, 4, 128], F32, name="at_ps")
                    for sub4 in range(4):
                        nc.tensor.matmul(
                            at_ps[:, sub4, :], emn[:, sub4, :], identb,
                            start=True, stop=True)
                    nc.scalar.copy(
                        out=at72[:, m0 + g * 512:m0 + (g + 1) * 512],
                        in_=at_ps.rearrange("p s q -> p (s q)"))
                    # store this group's half of at_d right away so back()
                    # can begin its broadcasts before the whole chunk is done
                    # layout: at_d[hh*9+kk, ch*2048 + g*1024 + jj*512 + m]
                    for jj in range(2):
                        qeng = [nc.sync, nc.scalar][(g + jj) % 2]
                        qeng.dma_start(
                            out=at_d[0:36, 2 * m0 + g * MCH + jj * 512:
                                     2 * m0 + g * MCH + (jj + 1) * 512],
                            in_=at72[36 * jj:36 * (jj + 1),
                                     m0 + g * 512:m0 + (g + 1) * 512])


            def back(ch, m0=None, width=None):
                """broadcast attn, in-place attn*v, transpose-accum, DMA."""
                if m0 is None:
                    m0 = ch * MCH
                if width is None:
                    width = MCH
                ngrp = width // 512
                last = (ch == NCH - 1)
                bc_t = [None] * K2
                ch0 = m0 // MCH
                gof = (m0 % MCH) // 512
                for kk in range(K2):
                    bc = bsb.tile([128, 2 * width], BF16, name=f"bcd{kk}")
                    base = 2 * ch0 * MCH + gof * MCH
                    src = at_d[kk:kk + 28:9, base:base + 2 * width]
                    src = src.unsqueeze(1).broadcast_to([4, 32, 2 * width])
                    qeng = [nc.sync, nc.gpsimd, nc.sync][kk % 3]
                    qeng.dma_start(out=bc, in_=src)
                    bc_t[kk] = bc
                o_ps = {g: back.ops.tile([128, 2, 4, 128], F32, name=f"o_ps{g}")
                        for g in range(ngrp)}
                # explicitly zero each bank with a full-footprint start=True
                # matmul (hardware zeroes per instruction footprint, not per
                # bank, so the first write of every byte must be a start)
                for g in range(ngrp):
                    for j in range(2):
                        nc.tensor.matmul(
                            o_ps[g][:, j, :, :].rearrange("p s q -> p (s q)"),
                            warm[:, 0:128], warm, start=True, stop=False)
                for kk in range(K2):
                    dl = DELTAS[kk]
                    eng = nc.gpsimd if kk in _AV_POOL else nc.vector
                    vsl = vT[:, :, PAD + m0 + dl:PAD + m0 + width + dl]
                    if width == MCH:
                        # bc layout [c, g, jj, 512]
                        bcv = bc_t[kk].rearrange("p (g jj m) -> p g jj m",
                                                 g=2, jj=2)
                        eng.tensor_mul(
                            bcv, bcv,
                            vsl.rearrange("p jj (g m) -> p g jj m", g=2))
                    else:
                        # bc layout [c, jj, 512]
                        bcv = bc_t[kk].rearrange("p (jj m) -> p jj m", jj=2)
                        eng.tensor_mul(bcv, bcv, vsl)
                    for j in range(2):
                        for g in range(ngrp):
                            for sub4 in range(4):
                                if width == MCH:
                                    lhs = bc_t[kk].rearrange(
                                        "p (g jj m) -> p g jj m", g=2, jj=2)[
                                        :, g, j, sub4 * 128:(sub4 + 1) * 128]
                                else:
                                    lhs = bc_t[kk].rearrange(
                                        "p (jj m) -> p jj m", jj=2)[
                                        :, j, sub4 * 128:(sub4 + 1) * 128]
                                nc.tensor.matmul(
                                    o_ps[g][:, j, sub4, :], lhs, identb,
                                    start=False,
                                    stop=(kk == K2 - 1 and sub4 == 3))
                for g in range(ngrp):
                    o_sb = back.osb.tile([128, 2, 4, 128], F32, name="o_sb")
                    if last and g % 2 == 1:
                        nc.vector.tensor_copy(out=o_sb, in_=o_ps[g])
                    else:
                        nc.scalar.copy(out=o_sb, in_=o_ps[g])
                    s0 = m0 // 128 + g * 4
                    for j in range(2):
                        qeng = [nc.sync, nc.scalar, nc.gpsimd][(g * 2 + j) % 3] \
                            if last else [nc.sync, nc.scalar][(g + j) % 2]
                        qeng.dma_start(
                            out=out_view[:, s0:s0 + 4, j, :],
                            in_=o_sb[:, j, :, :])

            with tc.tile_pool(name="qkv_ps", bufs=2, space="PSUM") as qps:
                project.qps = qps
                rr = ['a']
                project([(fam, mch) for mch in range(2) for fam in range(2)], rr)
                t0 = [None] * K2
                front_products(0, range(3), t0)
                project([(fam, 2) for fam in range(2)], rr)
                front(0, t_t=t0, kks=range(3, K2))
                project([(fam, mch) for mch in range(3, 8) for fam in range(2)], rr)
                project([(2, mch) for mch in range(8)], rr)
            xt_pool.release()
            with tc.tile_pool(name="o_ps", bufs=1, space="PSUM") as ops, \
                 tc.tile_pool(name="o_sb", bufs=2) as osb:
                back.ops = ops
                back.osb = osb
                for ch in range(1, NCH):
                    front(ch)
                    back(ch - 1)
                back(NCH - 1, m0=(NCH - 1) * MCH, width=512)
                back(NCH - 1, m0=(NCH - 1) * MCH + 512, width=512)
    nc.compile()
    return nc


_NC_CACHE = None


def make_in_map(x_core: np.ndarray, W_qkv: np.ndarray) -> dict:
    """Per-core input dict. x_core: [N, C] f32, W_qkv: [768, 256] f32."""
    consts = dict(zip(_CONST_NAMES, _host_consts()))
    return {
        "x": np.ascontiguousarray(x_core.reshape(N, C)).astype(NPBF16),
        "w": np.ascontiguousarray(W_qkv).astype(NPBF16),
        **consts,
    }


def kernel(x: np.ndarray, W_qkv: np.ndarray) -> np.ndarray:
    global _NC_CACHE
    if _NC_CACHE is None:
        _NC_CACHE = build_nc()
    nc = _NC_CACHE

    x = np.ascontiguousarray(x, dtype=np.float32)
    W_qkv = np.ascontiguousarray(W_qkv, dtype=np.float32)
    in_maps = [make_in_map(x[b], W_qkv) for b in range(B)]
    res = run_bass_kernel_spmd(nc, in_maps, list(range(B)))
    out = np.stack([res.results[b]["out"].reshape(H, W, C) for b in range(B)])
    return out


if __name__ == "__main__":
    rng = np.random.default_rng(0)
    x = rng.standard_normal((B, H, W, C), dtype=np.float32)
    wq = (rng.standard_normal((3 * C, C), dtype=np.float32) * 0.02).astype(np.float32)
    out = kernel(x, wq)
    print("out", out.shape, out.dtype, float(np.abs(out).mean()))
